# revision 8
# baseline (speedup 1.0000x reference)
"""Trainium2 Bass kernel for DifferentiableSoftmaxTree NLL (hierarchical
softmax negative log-likelihood).

Math: the 2-way log_softmax at each tree node reduces to a softplus of a
logit difference, so for sample b with path nodes n_k / directions d_k:
    s_k  = features[b] . (node_weights[n_k,:,1] - node_weights[n_k,:,0])
    out[b] = sum_k mask_k * softplus((1-2 d_k) * s_k)

Strategy (data-parallel over batch, 8 cores x 512 samples, 4 blocks of 128):

  TOP LEVELS (0..8, heap node ids 0..510): every sample visits all 9 of
  these levels, so instead of gathering per-sample weights we matmul the
  block's features against ALL 511 node weight-diff columns on the PE
  (fp16, 4 contraction chunks of 128 into one PSUM bank -> [128,511]
  fp32 logits). Selection of the 9 path logits per sample is a signed
  multi-hot built with 9 DVE tensor_scalar ops (iota == node_id) * sign,
  then one tensor_tensor multiply against PSUM. Non-path entries become
  exactly 0, and softplus(0)=ln2 is a constant the host folds into a
  per-sample correction, so no per-level masking is needed.

  DEEP LEVELS (9..15): per-sample indirect (SWDGE) gather of one
  contiguous fp16 row per sample from a host-built table
  dtab[class] = concat_k(wdiff[n_k(class)]) of shape [50000, 7*512]
  (masked levels zeroed -> contribute softplus(0), also folded into the
  host-side correction). 7KB/sample instead of the 32KB fp32 full-path
  row: 4.6x less gather traffic. Dot products vs features on DVE
  (fp16 multiply in-place + one batched tensor_reduce).

  SOFTPLUS of all 518 terms in 2 ACT ops: Exp(u) then Ln(e+1) with
  accum_out summing the row (|u| <~ 12 here so exp can't overflow), then
  subtract the host correction (518 - pathlen)*ln2.

  (tensor_tensor_reduce is avoided: it wedges this runtime. Multi-offset
  indirect DMA is avoided: HW honours only the first offset per
  partition.)
"""

import numpy as np
from contextlib import ExitStack

import concourse.bass as bass
import concourse.mybir as mybir
import concourse.tile as tile
from concourse import bass_utils
import concourse.bacc as bacc

NUM_CLASSES = 50000
NUM_INTERNAL = NUM_CLASSES - 1
D = 512
B = 4096
K = 16
N_CORES = 8
BL = B // N_CORES          # samples per core
P = 128                    # partition dim
NBLK = BL // P             # 128-sample blocks per core
JTOP = 9                   # tree levels computed via PE matmul
NTOP = (1 << JTOP) - 1     # 511 heap nodes in levels 0..8
KD = K - JTOP              # 7 deep levels gathered per sample
NU = NTOP + KD             # softplus terms per sample
LN2 = float(np.log(2.0))

_AF = mybir.ActivationFunctionType
_OP = mybir.AluOpType
_F16 = mybir.dt.float16
_F32 = mybir.dt.float32
_I32 = mybir.dt.int32

# meta int32 word layout (28 words / 112B per sample; all-but-w0 are fp32
# bit patterns -- tensor_scalar requires fp32 scalar operands)
#   w0       : target class id (gather row index)
#   w1       : correction c_b = (NU - pathlen_b) * ln2
#   w2..w10  : top node ids per level j=0..8
#   w11..w19 : top signs (1-2*dir)
#   w20..w26 : deep masked signs (mask * (1-2*dir))
MW = 28


def _build_program():
    nc = bacc.Bacc(
        "TRN2",
        target_bir_lowering=False,
        debug=False,
        enable_asserts=False,
        num_devices=N_CORES,
    )
    feat_ap = nc.dram_tensor("feat", [BL, D], _F16, kind="ExternalInput").ap()
    featT_ap = nc.dram_tensor("featT", [BL, D], _F16, kind="ExternalInput").ap()
    wtopT_ap = nc.dram_tensor("wtopT", [P, 4 * NTOP], _F16, kind="ExternalInput").ap()
    iota_ap = nc.dram_tensor("iota", [P, NTOP], _F32, kind="ExternalInput").ap()
    meta_ap = nc.dram_tensor("meta", [BL, MW], _I32, kind="ExternalInput").ap()
    dtab_ap = nc.dram_tensor("dtab", [NUM_CLASSES, KD * D], _F16, kind="ExternalInput").ap()
    out_ap = nc.dram_tensor("out", [BL, 1], _F32, kind="ExternalOutput").ap()

    with tile.TileContext(nc) as tc, ExitStack() as ctx:
        const_pool = ctx.enter_context(tc.tile_pool(name="const", bufs=1))
        feat_pool = ctx.enter_context(tc.tile_pool(name="feat", bufs=2))
        ft_pool = ctx.enter_context(tc.tile_pool(name="ft", bufs=2))
        meta_pool = ctx.enter_context(tc.tile_pool(name="meta", bufs=2))
        gath_pool = ctx.enter_context(tc.tile_pool(name="gath", bufs=3))
        mh_pool = ctx.enter_context(tc.tile_pool(name="mh", bufs=2))
        u_pool = ctx.enter_context(tc.tile_pool(name="u", bufs=2))
        e_pool = ctx.enter_context(tc.tile_pool(name="e", bufs=2))
        dump_pool = ctx.enter_context(tc.tile_pool(name="dump", bufs=2))
        small_pool = ctx.enter_context(tc.tile_pool(name="small", bufs=2))
        psum_pool = ctx.enter_context(tc.tile_pool(name="psum", bufs=2, space="PSUM"))

        wt_t = const_pool.tile([P, 4 * NTOP], _F16, tag="wt")
        nc.sync.dma_start(wt_t[:], wtopT_ap[:])
        iota_t = const_pool.tile([P, NTOP], _F32, tag="iota")
        nc.sync.dma_start(iota_t[:], iota_ap[:])

        for blk in range(NBLK):
            b0 = blk * P
            meta_t = meta_pool.tile([P, MW], _I32, tag="meta")
            nc.sync.dma_start(meta_t[:], meta_ap[b0 : b0 + P, :])
            feat_t = feat_pool.tile([P, D], _F16, tag="feat")
            nc.sync.dma_start(feat_t[:], feat_ap[b0 : b0 + P, :])
            ft_t = ft_pool.tile([P, D], _F16, tag="ft")
            nc.sync.dma_start(ft_t[:], featT_ap[b0 : b0 + P, :])

            # per-sample gather of the 7 deep-level weight rows (one 7KB row)
            g_t = gath_pool.tile([P, KD * D], _F16, tag="g")
            nc.gpsimd.indirect_dma_start(
                out=g_t[:],
                out_offset=None,
                in_=dtab_ap[:],
                in_offset=bass.IndirectOffsetOnAxis(ap=meta_t[:, 0:1], axis=0),
            )

            # all 511 top-level logits for the block: featT.T @ wtopT
            ps_t = psum_pool.tile([P, NTOP], _F32, tag="ps")
            for c in range(4):
                nc.tensor.matmul(
                    ps_t[:],
                    lhsT=ft_t[:, c * P : (c + 1) * P],
                    rhs=wt_t[:, c * NTOP : (c + 1) * NTOP],
                    start=(c == 0),
                    stop=(c == 3),
                )

            meta32 = meta_t[:].bitcast(_F32)        # [P, MW]

            # signed multi-hot over the 511 top nodes: (iota==id_j) * sgn_j
            mh_t = mh_pool.tile([P, NTOP], _F16, tag="mh")
            for j in range(JTOP):
                lo = (1 << j) - 1
                hi = (2 << j) - 1
                nc.vector.tensor_scalar(
                    out=mh_t[:, lo:hi],
                    in0=iota_t[:, lo:hi],
                    scalar1=meta32[:, 2 + j : 3 + j],
                    scalar2=meta32[:, 11 + j : 12 + j],
                    op0=_OP.is_equal,
                    op1=_OP.mult,
                )

            u_t = u_pool.tile([P, NU], _F32, tag="u")
            nc.vector.tensor_tensor(
                out=u_t[:, 0:NTOP], in0=mh_t[:], in1=ps_t[:], op=_OP.mult
            )

            # deep levels: in-place multiply by features, reduce per level
            g3 = g_t[:].rearrange("p (k d) -> p k d", k=KD)
            nc.vector.tensor_tensor(
                out=g3,
                in0=g3,
                in1=feat_t[:, None, :].to_broadcast([P, KD, D]),
                op=_OP.mult,
            )
            s_t = small_pool.tile([P, KD], _F32, tag="s")
            nc.vector.tensor_reduce(
                out=s_t[:], in_=g3, axis=mybir.AxisListType.X, op=_OP.add
            )
            nc.vector.tensor_tensor(
                out=u_t[:, NTOP:NU], in0=s_t[:], in1=meta32[:, 20 : 20 + KD],
                op=_OP.mult,
            )

            # sum softplus(u) over all 518 terms, then subtract correction
            e_t = e_pool.tile([P, NU], _F32, tag="e")
            nc.scalar.activation(e_t[:], u_t[:], _AF.Exp)
            d_t = dump_pool.tile([P, NU], _F32, tag="d")
            acc_t = small_pool.tile([P, 1], _F32, tag="acc")
            nc.scalar.activation(d_t[:], e_t[:], _AF.Ln, bias=1.0, accum_out=acc_t[:])
            res_t = small_pool.tile([P, 1], _F32, tag="res")
            nc.vector.tensor_scalar(
                out=res_t[:],
                in0=acc_t[:],
                scalar1=meta32[:, 1:2],
                scalar2=None,
                op0=_OP.subtract,
            )
            nc.sync.dma_start(out_ap[b0 : b0 + P, :], res_t[:])

    nc.compile()
    return nc


_PROGRAM_CACHE = {}


def _get_program():
    if "nc" not in _PROGRAM_CACHE:
        _PROGRAM_CACHE["nc"] = _build_program()
    return _PROGRAM_CACHE["nc"]


def _reset_device():
    # A previously-crashed kernel can leave an exec unit wedged; a
    # client-side axon reset clears it and is near-free otherwise.
    try:
        import ctypes

        lib = ctypes.CDLL("/opt/axon/libaxon_pjrt.so")
        lib.axon_reset.restype = ctypes.c_int64
        lib.axon_reset()
    except Exception:
        pass


def _prepare_inputs(features, targets, node_weights, path_nodes_map, path_directions_map):
    features = np.asarray(features, dtype=np.float32)
    targets = np.asarray(targets, dtype=np.int32)
    node_weights = np.asarray(node_weights, dtype=np.float32)
    path_nodes_map = np.asarray(path_nodes_map, dtype=np.int32)
    path_directions_map = np.asarray(path_directions_map, dtype=np.int32)

    wdiff = node_weights[:, :, 1] - node_weights[:, :, 0]     # [N_INT, D] f32
    maskmap = path_nodes_map != -1                             # [C, K]

    # deep-level per-class table [C, KD*D] fp16, masked levels zeroed
    deep_nodes = np.where(maskmap[:, JTOP:], path_nodes_map[:, JTOP:], 0)
    dtab = wdiff.astype(np.float16)[deep_nodes]                # [C, KD, D]
    dtab[~maskmap[:, JTOP:]] = np.float16(0.0)
    dtab = np.ascontiguousarray(dtab.reshape(NUM_CLASSES, KD * D))

    # top-level weight matrix, chunked for the PE:
    # wtopT[p, c*NTOP + n] = wdiff[n, c*128 + p]
    wt = wdiff[:NTOP].astype(np.float16)                       # [511, 512]
    wtopT = np.ascontiguousarray(
        wt.reshape(NTOP, 4, P).transpose(2, 1, 0).reshape(P, 4 * NTOP)
    )

    iota = np.ascontiguousarray(
        np.broadcast_to(np.arange(NTOP, dtype=np.float32), (P, NTOP))
    )

    # per-sample metadata
    tflat = targets.reshape(-1)
    bnodes = path_nodes_map[tflat]                             # [B, K]
    bdirs = path_directions_map[tflat]
    bmask = maskmap[tflat]
    pathlen = bmask.sum(axis=1).astype(np.int32)               # 15 or 16
    sgn = (1 - 2 * bdirs).astype(np.float32)                   # [B, K]
    msgn_deep = np.where(bmask[:, JTOP:], sgn[:, JTOP:], np.float32(0.0))
    corr = ((NU - pathlen).astype(np.float32) * np.float32(LN2))

    metaf = np.zeros((B, MW), dtype=np.float32)
    metaf[:, 2:11] = bnodes[:, :JTOP].astype(np.float32)
    metaf[:, 11:20] = sgn[:, :JTOP]
    metaf[:, 20 : 20 + KD] = msgn_deep
    metaf[:, 1] = corr
    meta = metaf.view(np.int32)
    meta[:, 0] = tflat
    meta = np.ascontiguousarray(meta)

    feat16 = features.astype(np.float16)                       # [B, D]

    in_maps = []
    for i in range(N_CORES):
        fc = feat16[i * BL : (i + 1) * BL]                     # [BL, D]
        # featT[blk*128+p, c*128+i] = fc[blk*128+i, c*128+p]
        ftT = np.ascontiguousarray(
            fc.reshape(NBLK, P, 4, P).transpose(0, 3, 2, 1).reshape(BL, D)
        )
        in_maps.append(
            {
                "feat": np.ascontiguousarray(fc),
                "featT": ftT,
                "wtopT": wtopT,
                "iota": iota,
                "meta": meta[i * BL : (i + 1) * BL],
                "dtab": dtab,
            }
        )
    return in_maps


def kernel(features, targets, node_weights, path_nodes_map, path_directions_map):
    in_maps = _prepare_inputs(
        features, targets, node_weights, path_nodes_map, path_directions_map
    )
    _reset_device()
    nc = _get_program()
    res = bass_utils.run_bass_kernel_spmd(nc, in_maps, core_ids=list(range(N_CORES)))
    out = np.concatenate([res.results[i]["out"].reshape(-1) for i in range(N_CORES)])
    return out.astype(np.float32)


# revision 11
# speedup vs baseline: 1.2308x; 1.2308x over previous
"""Trainium2 Bass kernel for DifferentiableSoftmaxTree NLL (hierarchical
softmax negative log-likelihood).

Math: the 2-way log_softmax at each tree node reduces to a softplus of a
logit difference, so for sample b with path nodes n_k / directions d_k:
    s_k  = features[b] . (node_weights[n_k,:,1] - node_weights[n_k,:,0])
    out[b] = sum_k mask_k * softplus((1-2 d_k) * s_k)

Strategy (data-parallel over batch, 8 cores x 512 samples, 4 blocks of 128):

  TOP LEVELS (0..8, heap node ids 0..510): every sample visits all 9 of
  these levels, so the device matmuls the block's features against ALL
  511 node weight-diff columns on the PE (fp16, 4 contraction chunks of
  128 into one PSUM bank -> [128,511] fp32 logits). The host sends a
  signed multi-hot mask mh[b,n] = sign_j at the 9 path nodes (0
  elsewhere); one DVE tensor_tensor against PSUM yields u = sign*logit
  at path nodes and exactly 0 elsewhere. softplus(0)=ln2 is constant, so
  a per-sample correction (NU - pathlen)*ln2 fixes the sum -- no
  per-level masking on device.

  DEEP LEVELS (9..15): the HOST pre-gathers each sample's 7 deep-level
  weight-diff rows into a dense [BL, 7*512] fp16 stream (a measured
  on-device SWDGE gather pays ~570ns of fixed cost PER ROW -> ~100GB/s;
  a dense HWDGE read of the same bytes runs at HBM line rate). Dot
  products vs features on DVE (fp16 multiply in-place + per-level
  reduction).

  All other per-sample operands (feat fp16, featT fp16 for the PE, mh
  fp16, meta) are packed into ONE [128, PKW] int32 row per sample so each
  block issues a single HWDGE DMA besides the deep stream.

  SOFTPLUS of all 518 terms in 2 ACT ops: Exp(u) then Ln(e+1) with
  accum_out summing the row (|u| <~ 12 here so exp can't overflow), then
  subtract the host correction. (Exp and Ln live in different ACT
  table-sets on this runtime -- each switch costs ~1.3us -- so a direct
  Softplus table is used instead when available.)

  (tensor_tensor_reduce is avoided: it wedges this runtime.)
"""

import numpy as np
from contextlib import ExitStack

import concourse.bass as bass
import concourse.mybir as mybir
import concourse.tile as tile
from concourse import bass_utils
import concourse.bacc as bacc

NUM_CLASSES = 50000
NUM_INTERNAL = NUM_CLASSES - 1
D = 512
B = 4096
K = 16
N_CORES = 8
BL = B // N_CORES          # samples per core
P = 128                    # partition dim
NBLK = BL // P             # 128-sample blocks per core
JTOP = 9                   # tree levels computed via PE matmul
NTOP = (1 << JTOP) - 1     # 511 heap nodes in levels 0..8
KD = K - JTOP              # 7 deep levels gathered per sample
NU = NTOP + KD             # softplus terms per sample
LN2 = float(np.log(2.0))

# tuning flags (settled by probe measurements):
# - Softplus activation table is a different function on this runtime
#   (probe: max abs err 36 vs log1p(exp)) -> Exp+Ln pair.
# - tensor_reduce runs at 1x mode regardless of dtype/shape (probe: 673ns
#   per [128,512] fp16 level, 3.87us for the 3D form) -> tree-fold the
#   512-wide levels down to 64 with 2x-mode fp16 tensor_tensor adds, then
#   one small 3D reduce.
USE_SOFTPLUS = False
FOLD_TO = 64               # level width after TT tree-folds (then 3D reduce)

_AF = mybir.ActivationFunctionType
_OP = mybir.AluOpType
_F16 = mybir.dt.float16
_F32 = mybir.dt.float32
_I32 = mybir.dt.int32

# packed per-sample input row, int32 words:
#   fp16[0:512)     = features
#   fp16[512:1024)  = featT block rows (d-major chunks for the PE)
#   fp16[1024:1535) = mh signed multi-hot over top nodes (fp16[1535] pad)
#   w769            = fp32 correction (NU - pathlen)*ln2
#   w770..776       = fp32 deep masked signs
PKW = 796


def _build_program():
    nc = bacc.Bacc(
        "TRN2",
        target_bir_lowering=False,
        debug=False,
        enable_asserts=False,
        num_devices=N_CORES,
    )
    pk_ap = nc.dram_tensor("pk", [BL, PKW], _I32, kind="ExternalInput").ap()
    pdeep_ap = nc.dram_tensor("pdeep", [BL, KD * D], _F16, kind="ExternalInput").ap()
    wtopT_ap = nc.dram_tensor("wtopT", [P, 4 * NTOP], _F16, kind="ExternalInput").ap()
    out_ap = nc.dram_tensor("out", [BL, 1], _F32, kind="ExternalOutput").ap()

    with tile.TileContext(nc) as tc, ExitStack() as ctx:
        const_pool = ctx.enter_context(tc.tile_pool(name="const", bufs=1))
        pk_pool = ctx.enter_context(tc.tile_pool(name="pk", bufs=3))
        deep_pool = ctx.enter_context(tc.tile_pool(name="deep", bufs=3))
        u_pool = ctx.enter_context(tc.tile_pool(name="u", bufs=2))
        e_pool = ctx.enter_context(tc.tile_pool(name="e", bufs=2))
        dump_pool = ctx.enter_context(tc.tile_pool(name="dump", bufs=2))
        small_pool = ctx.enter_context(tc.tile_pool(name="small", bufs=2))
        psum_pool = ctx.enter_context(tc.tile_pool(name="psum", bufs=2, space="PSUM"))

        wt_t = const_pool.tile([P, 4 * NTOP], _F16, tag="wt")
        nc.sync.dma_start(wt_t[:], wtopT_ap[:])

        for blk in range(NBLK):
            b0 = blk * P
            pk_t = pk_pool.tile([P, PKW], _I32, tag="pk")
            nc.sync.dma_start(pk_t[:], pk_ap[b0 : b0 + P, :])
            g_t = deep_pool.tile([P, KD * D], _F16, tag="g")
            nc.sync.dma_start(g_t[:], pdeep_ap[b0 : b0 + P, :])

            pk16 = pk_t[:].bitcast(_F16)        # [P, 2*PKW]
            pk32 = pk_t[:].bitcast(_F32)        # [P, PKW]

            # all 511 top-level logits for the block: featT.T @ wtopT
            ps_t = psum_pool.tile([P, NTOP], _F32, tag="ps")
            for c in range(4):
                nc.tensor.matmul(
                    ps_t[:],
                    lhsT=pk16[:, 512 + c * P : 512 + (c + 1) * P],
                    rhs=wt_t[:, c * NTOP : (c + 1) * NTOP],
                    start=(c == 0),
                    stop=(c == 3),
                )

            u_t = u_pool.tile([P, NU], _F32, tag="u")
            nc.vector.tensor_tensor(
                out=u_t[:, 0:NTOP], in0=pk16[:, 1024 : 1024 + NTOP], in1=ps_t[:],
                op=_OP.mult,
            )

            # deep levels: in-place multiply by features, reduce per level
            g3 = g_t[:].rearrange("p (k d) -> p k d", k=KD)
            nc.vector.tensor_tensor(
                out=g3,
                in0=g3,
                in1=pk16[:, 0:D][:, None, :].to_broadcast([P, KD, D]),
                op=_OP.mult,
            )
            w = D
            while w > FOLD_TO:
                h = w // 2
                nc.vector.tensor_tensor(
                    out=g_t[:].rearrange("p (k d) -> p k d", k=KD)[:, :, 0:h],
                    in0=g_t[:].rearrange("p (k d) -> p k d", k=KD)[:, :, 0:h],
                    in1=g_t[:].rearrange("p (k d) -> p k d", k=KD)[:, :, h:w],
                    op=_OP.add,
                )
                w = h
            s_t = small_pool.tile([P, KD], _F32, tag="s")
            nc.vector.tensor_reduce(
                out=s_t[:],
                in_=g_t[:].rearrange("p (k d) -> p k d", k=KD)[:, :, 0:FOLD_TO],
                axis=mybir.AxisListType.X,
                op=_OP.add,
            )
            nc.vector.tensor_tensor(
                out=u_t[:, NTOP:NU], in0=s_t[:], in1=pk32[:, 770 : 770 + KD],
                op=_OP.mult,
            )

            # sum softplus(u) over all 518 terms, then subtract correction
            acc_t = small_pool.tile([P, 1], _F32, tag="acc")
            d_t = dump_pool.tile([P, NU], _F32, tag="d")
            if USE_SOFTPLUS:
                nc.scalar.activation(
                    d_t[:], u_t[:], _AF.Softplus, accum_out=acc_t[:]
                )
            else:
                e_t = e_pool.tile([P, NU], _F32, tag="e")
                nc.scalar.activation(e_t[:], u_t[:], _AF.Exp)
                nc.scalar.activation(
                    d_t[:], e_t[:], _AF.Ln, bias=1.0, accum_out=acc_t[:]
                )
            res_t = small_pool.tile([P, 1], _F32, tag="res")
            nc.vector.tensor_scalar(
                out=res_t[:],
                in0=acc_t[:],
                scalar1=pk32[:, 769:770],
                scalar2=None,
                op0=_OP.subtract,
            )
            nc.sync.dma_start(out_ap[b0 : b0 + P, :], res_t[:])

    nc.compile()
    return nc


_PROGRAM_CACHE = {}


def _get_program():
    if "nc" not in _PROGRAM_CACHE:
        _PROGRAM_CACHE["nc"] = _build_program()
    return _PROGRAM_CACHE["nc"]


def _reset_device():
    # A previously-crashed kernel can leave an exec unit wedged; a
    # client-side axon reset clears it and is near-free otherwise.
    try:
        import ctypes

        lib = ctypes.CDLL("/opt/axon/libaxon_pjrt.so")
        lib.axon_reset.restype = ctypes.c_int64
        lib.axon_reset()
    except Exception:
        pass


def _prepare_inputs(features, targets, node_weights, path_nodes_map, path_directions_map):
    features = np.asarray(features, dtype=np.float32)
    targets = np.asarray(targets, dtype=np.int32)
    node_weights = np.asarray(node_weights, dtype=np.float32)
    path_nodes_map = np.asarray(path_nodes_map, dtype=np.int32)
    path_directions_map = np.asarray(path_directions_map, dtype=np.int32)

    wdiff = node_weights[:, :, 1] - node_weights[:, :, 0]     # [N_INT, D] f32
    maskmap = path_nodes_map != -1                             # [C, K]
    wdiff16 = wdiff.astype(np.float16)

    # top-level weight matrix, chunked for the PE:
    # wtopT[p, c*NTOP + n] = wdiff[n, c*128 + p]
    wtopT = np.ascontiguousarray(
        wdiff16[:NTOP].reshape(NTOP, 4, P).transpose(2, 1, 0).reshape(P, 4 * NTOP)
    )

    # per-sample metadata
    tflat = targets.reshape(-1)
    bnodes = path_nodes_map[tflat]                             # [B, K]
    bdirs = path_directions_map[tflat]
    bmask = maskmap[tflat]
    pathlen = bmask.sum(axis=1).astype(np.int32)               # 15 or 16
    sgn = (1 - 2 * bdirs).astype(np.float32)                   # [B, K]
    msgn_deep = np.where(bmask[:, JTOP:], sgn[:, JTOP:], np.float32(0.0))
    corr = (NU - pathlen).astype(np.float32) * np.float32(LN2)

    # signed multi-hot over the 511 top nodes
    mh = np.zeros((B, NTOP + 1), dtype=np.float16)
    rows = np.arange(B)
    for j in range(JTOP):
        mh[rows, bnodes[:, j]] = sgn[:, j].astype(np.float16)
    mh = mh[:, :NTOP]

    # host pre-gather of each sample's deep-level rows (masked levels zero)
    deep_nodes = np.where(bmask[:, JTOP:], bnodes[:, JTOP:], 0)   # [B, KD]
    pdeep = wdiff16[deep_nodes]                                   # [B, KD, D]
    pdeep[~bmask[:, JTOP:]] = np.float16(0.0)
    pdeep = np.ascontiguousarray(pdeep.reshape(B, KD * D))

    feat16 = features.astype(np.float16)                          # [B, D]

    in_maps = []
    for i in range(N_CORES):
        sl = slice(i * BL, (i + 1) * BL)
        fc = feat16[sl]                                           # [BL, D]
        # featT[blk*128+p, c*128+i] = fc[blk*128+i, c*128+p]
        ftT = fc.reshape(NBLK, P, 4, P).transpose(0, 3, 2, 1).reshape(BL, D)

        pk = np.zeros((BL, PKW), dtype=np.int32)
        pk16 = pk.view(np.float16)                                # [BL, 2*PKW]
        pk32 = pk.view(np.float32)                                # [BL, PKW]
        pk16[:, 0:D] = fc
        pk16[:, D : 2 * D] = ftT
        pk16[:, 1024 : 1024 + NTOP] = mh[sl]
        pk32[:, 769] = corr[sl]
        pk32[:, 770 : 770 + KD] = msgn_deep[sl]

        in_maps.append(
            {
                "pk": np.ascontiguousarray(pk),
                "pdeep": pdeep[sl],
                "wtopT": wtopT,
            }
        )
    return in_maps


def kernel(features, targets, node_weights, path_nodes_map, path_directions_map):
    in_maps = _prepare_inputs(
        features, targets, node_weights, path_nodes_map, path_directions_map
    )
    _reset_device()
    nc = _get_program()
    res = bass_utils.run_bass_kernel_spmd(nc, in_maps, core_ids=list(range(N_CORES)))
    out = np.concatenate([res.results[i]["out"].reshape(-1) for i in range(N_CORES)])
    return out.astype(np.float32)
